# revision 29
# baseline (speedup 1.0000x reference)
"""GCNConv on 8 Trainium2 NeuronCores.

out = D^-1/2 (A + I) D^-1/2 (x @ W.T + b)

Strategy (dest-bucket sharding):
  - algebra: norm[e] = dis[row]*dis[col] folds into node scalings:
        g = dis * (x @ W.T + b)          (per-node, on device)
        out[d] = dis[d] * sum_{e: row=d} g[col_e]
  - host: append self loops, bucket edges by dest core (6250 dests/core),
    sort dests by degree into windows of 128 (one dest per SBUF
    partition), build int16 gather grids for SWDGE dma_gather. int16
    indices only span 32768 table rows, so three overlapping base regions
    are used; each dest's edges are split ~evenly across the three grids
    (cols sorted ascending; contiguous ranges stay region-valid).
  - device (SPMD x8): phase A = g table via PE matmul (bias preloaded
    into PSUM by the scalar engine, 4 node tiles per PSUM bank, one DVE
    scale+cast per tile); phase B = dma_gather of g rows into
    [128, C_w, 128] tiles (one dest per partition), DVE reduce over the
    slot axis, scale by dis[dest], store.
  - host: inverse-permute the [win, part] layout back to node order.

g table layout (GT_ROWS = NPAD + 384):
  row 0..127            zeros
  row n+128             g[node n]          for n < 25024
  row 25152..25279      zeros
  row n+256             g[node n]          for n >= 25024
  row NPAD+256..+383    zeros
Gather base regions (int16 idx = row - base):
  G1 base 0     -> nodes <= 32511   (pad idx 0)
  G2 base 8832  -> nodes in [8704, 41343]   (pad idx 16320)
  G3 base 17664 -> nodes >= 17536   (pad idx 32640)
"""

import numpy as np

N_NODES = 50000
N_EDGES = 1600000
IN_CH = 256
OUT_CH = 128
N_CORES = 8

DPC = N_NODES // N_CORES          # dests per core
WPC = (DPC + 127) // 128          # windows per core
NPAD = ((N_NODES + 127) // 128) * 128   # padded node count
NT = NPAD // 128                  # node tiles for the matmul
GT_ROWS = NPAD + 384
MIDSPLIT = 25024                  # node where the middle zero band sits
G2BASE = 8832
G3BASE = 17664
G1MAX = 32511                     # max node reachable from base 0
G2MIN, G2MAX = 8704, 41343
G3MIN = 17536
G2PAD = 16320
G3PAD = 32640
GCHUNK_COLS = 16                  # dma_gather chunk: 16 cols * 128 = 2048 idxs
XSLAB_T = 23                      # node tiles per x slab (23*17 = 391 = NT)
PGROUP = 4                        # node tiles per PSUM bank


def _row_of(n):
    n = np.asarray(n, dtype=np.int64)
    return n + 128 + 128 * (n >= MIDSPLIT)


def _plan(edge_index):
    """Host-side index preprocessing. Returns per-core gather grids and the
    permutation needed to unshard."""
    ei0 = np.asarray(edge_index[0], dtype=np.int64)
    ei1 = np.asarray(edge_index[1], dtype=np.int64)
    self_idx = np.arange(N_NODES, dtype=np.int64)
    row = np.concatenate([ei0, self_idx])
    col = np.concatenate([ei1, self_idx])

    deg = np.bincount(row, minlength=N_NODES)
    n1o = np.bincount(row[col < G2MIN], minlength=N_NODES)
    n3o = np.bincount(row[col > G2MAX], minlength=N_NODES)
    m1 = np.bincount(row[col <= G1MAX], minlength=N_NODES)
    m3 = np.bincount(row[col >= G3MIN], minlength=N_NODES)
    dis = deg.astype(np.float32) ** -0.5

    # 3-way split of each dest's (ascending) cols: first k1 -> G1,
    # middle k2 -> G2, last k3 -> G3; clip keeps every range region-valid.
    k1 = np.clip((deg + 2) // 3, n1o, m1)
    k3 = np.clip((deg - k1 + 1) // 2, n3o, np.minimum(m3, deg - k1))
    k2 = deg - k1 - k3

    order = np.lexsort((col, row))
    col_sorted = col[order].astype(np.int64)
    row_sorted = _row_of(col_sorted)  # table rows, in per-dest ascending order
    starts = np.zeros(N_NODES + 1, dtype=np.int64)
    np.cumsum(deg, out=starts[1:])

    # per-core window assignment: lex sort by (max k, total degree) desc —
    # keeps all three per-window column maxima tight simultaneously
    maxk = np.maximum(np.maximum(k1, k2), k3)
    perms = []
    kmax_pc = np.zeros((3, N_CORES, WPC), dtype=np.int64)
    for c in range(N_CORES):
        sl = slice(c * DPC, (c + 1) * DPC)
        perm = np.lexsort((-deg[sl], -maxk[sl]))
        perms.append(perm)
        for j, kk in enumerate((k1, k2, k3)):
            ks = kk[sl][perm]
            for w in range(WPC):
                s = w * 128
                e = min(s + 128, DPC)
                kmax_pc[j, c, w] = ks[s:e].max() if s < DPC else 0
    cks = kmax_pc.max(axis=1)          # [3, WPC]
    cw = cks.sum(axis=0)
    cw = np.maximum(cw, 1)
    offs = np.zeros(WPC + 1, dtype=np.int64)
    np.cumsum(cw, out=offs[1:])
    totc = int(offs[-1])

    idx_arrs, disw_arrs, gdests = [], [], []
    for c in range(N_CORES):
        perm = perms[c]
        grid = np.zeros((128, totc), dtype=np.int16)
        disw = np.zeros((128, WPC), dtype=np.float32)
        gdest = np.full((WPC, 128), -1, dtype=np.int64)
        for w in range(WPC):
            o = int(offs[w])
            c1, c2, c3 = int(cks[0, w]), int(cks[1, w]), int(cks[2, w])
            grid[:, o + c1:o + c1 + c2] = G2PAD
            grid[:, o + c1 + c2:o + c1 + c2 + c3] = G3PAD
            for p in range(128):
                s = w * 128 + p
                if s >= DPC:
                    break
                d = c * DPC + int(perm[s])
                a1, a2, a3 = int(k1[d]), int(k2[d]), int(k3[d])
                st = int(starts[d])
                rows_d = row_sorted[st:st + a1 + a2 + a3]
                grid[p, o:o + a1] = rows_d[:a1].astype(np.int16)
                grid[p, o + c1:o + c1 + a2] = \
                    (rows_d[a1:a1 + a2] - G2BASE).astype(np.int16)
                grid[p, o + c1 + c2:o + c1 + c2 + a3] = \
                    (rows_d[a1 + a2:] - G3BASE).astype(np.int16)
                disw[p, w] = dis[d]
                gdest[w, p] = d
        idx_arrs.append(grid)
        disw_arrs.append(disw)
        gdests.append(gdest)

    # wrap grids into the dma_gather idx layout: logical slot j
    # (= col*128 + part) lives at [16*g + j%16, j//16] for each of the 8
    # replicated 16-partition groups.
    idx16 = []
    tot16 = totc * 8
    for c in range(N_CORES):
        L = idx_arrs[c].T.ravel()
        base = L.reshape(tot16, 16).T
        idx16.append(np.ascontiguousarray(np.tile(base, (8, 1))))

    return {
        "dis": dis,
        "cks": cks,
        "cw": cw.astype(np.int64),
        "totc": totc,
        "tot16": tot16,
        "idx16": idx16,
        "disw": disw_arrs,
        "gdest": gdests,
    }


def _build_bass(cks, totc, tot16):
    """Build the single SPMD Bass program (same NEFF on all 8 cores)."""
    import concourse.bacc as bacc
    import concourse.mybir as mybir
    import concourse.tile as tile
    from concourse.library_config import mlp

    fp32 = mybir.dt.float32
    fp16 = mybir.dt.float16
    i16 = mybir.dt.int16

    nc = bacc.Bacc(
        "TRN2",
        target_bir_lowering=False,
        dynamic_dma_scratch_size=65536,
        num_swdge_queues=4,
    )

    xT = nc.dram_tensor("xT", [IN_CH, NPAD], fp16, kind="ExternalInput")
    wT = nc.dram_tensor("wT", [IN_CH, OUT_CH], fp16, kind="ExternalInput")
    bvec4 = nc.dram_tensor("bvec4", [128, PGROUP * OUT_CH], fp32,
                           kind="ExternalInput")
    disg = nc.dram_tensor("disg", [128, NT], fp32, kind="ExternalInput")
    disw = nc.dram_tensor("disw", [128, WPC], fp32, kind="ExternalInput")
    idx = nc.dram_tensor("idx", [128, tot16], i16, kind="ExternalInput")

    gtab = nc.dram_tensor("gtab", [GT_ROWS, OUT_CH], fp16, kind="Internal")
    outd = nc.dram_tensor("outd", [WPC, 128, OUT_CH], fp32, kind="ExternalOutput")

    cw = cks.sum(axis=0)
    offs = np.zeros(len(cw) + 1, dtype=np.int64)
    np.cumsum(cw, out=offs[1:])
    cmax = int(max(cw))

    with tile.TileContext(nc) as tc:
        # ---------------- phase A: g = dis * (x @ W.T + b) ----------------
        with (
            tc.tile_pool(name="constA", bufs=1) as cpool,
            tc.tile_pool(name="xslab", bufs=2) as xpool,
            tc.tile_pool(name="gout", bufs=4) as gpool,
            tc.tile_pool(name="psum", bufs=4, space="PSUM") as ppool,
        ):
            nc.gpsimd.load_library(mlp)
            wt0 = cpool.tile([128, OUT_CH], fp16, tag="wt0")
            wt1 = cpool.tile([128, OUT_CH], fp16, tag="wt1")
            bt4 = cpool.tile([128, PGROUP * OUT_CH], fp32, tag="bt4")
            dg = cpool.tile([128, NT], fp32, tag="dg")
            nc.sync.dma_start(wt0[:], wT[0:128, :])
            nc.sync.dma_start(wt1[:], wT[128:256, :])
            nc.sync.dma_start(bt4[:], bvec4[:])
            nc.sync.dma_start(dg[:], disg[:])

            # zero rows absorb padding gathers
            zt = cpool.tile([128, 3 * OUT_CH], fp16, tag="zt")
            nc.vector.memset(zt[:], 0.0)
            nc.sync.dma_start(gtab[0:128, :], zt[:, 0:OUT_CH])
            nc.sync.dma_start(gtab[MIDSPLIT + 128:MIDSPLIT + 256, :],
                              zt[:, OUT_CH:2 * OUT_CH])
            nc.sync.dma_start(gtab[NPAD + 256:GT_ROWS, :],
                              zt[:, 2 * OUT_CH:])

            nslab = NT // XSLAB_T
            for s in range(nslab):
                c0 = s * XSLAB_T * 128
                cols = XSLAB_T * 128
                xa = xpool.tile([128, cols], fp16, tag="xa")
                xb = xpool.tile([128, cols], fp16, tag="xb")
                nc.sync.dma_start(xa[:], xT[0:128, c0:c0 + cols])
                nc.sync.dma_start(xb[:], xT[128:256, c0:c0 + cols])
                t = 0
                while t < XSLAB_T:
                    gn = min(PGROUP, XSLAB_T - t)
                    ps = ppool.tile([128, PGROUP, OUT_CH], fp32, tag="ps")
                    gt = gpool.tile([128, PGROUP, OUT_CH], fp16, tag="gt")
                    gs = gpool.tile([128, PGROUP, OUT_CH], fp32, tag="gs")
                    for j in range(gn):
                        gt_i = s * XSLAB_T + t + j
                        sl = slice((t + j) * 128, (t + j + 1) * 128)
                        nc.tensor.matmul(
                            ps[:, j, :], xa[:, sl], wt0[:],
                            start=True, stop=False,
                        )
                        nc.tensor.matmul(
                            ps[:, j, :], xb[:, sl], wt1[:],
                            start=False, stop=True,
                        )
                        nc.vector.tensor_tensor(
                            gs[:, j, :], ps[:, j, :], bt4[:, 0:OUT_CH],
                            op=mybir.AluOpType.add,
                        )
                        nc.vector.tensor_scalar_mul(
                            gt[:, j, :], gs[:, j, :], dg[:, gt_i:gt_i + 1]
                        )
                        r0 = int(_row_of(gt_i * 128))
                        r1 = int(_row_of(gt_i * 128 + 64))
                        if r1 == r0 + 64:
                            nc.sync.dma_start(
                                gtab[r0:r0 + 128, :], gt[:, j, :]
                            )
                        else:  # tile straddling the middle zero band
                            nc.sync.dma_start(
                                gtab[r0:r0 + 64, :], gt[0:64, j, :]
                            )
                            nc.sync.dma_start(
                                gtab[r1:r1 + 64, :], gt[64:128, j, :]
                            )
                    t += gn

        # ---------------- phase B: gather + segment reduce ----------------
        with (
            tc.tile_pool(name="constB", bufs=1) as bpool,
            tc.tile_pool(name="msg", bufs=3) as mpool,
            tc.tile_pool(name="accp", bufs=2) as apool,
            tc.tile_pool(name="red", bufs=4) as rpool,
        ):
            dw = bpool.tile([128, WPC], fp32, tag="dw")
            nc.sync.dma_start(dw[:], disw[:])
            ix = bpool.tile([128, tot16], i16, tag="ix")
            nc.sync.dma_start(ix[:], idx[:])
            gq = 0

            for w in range(WPC):
                o = int(offs[w])
                c1, c2, c3 = int(cks[0, w]), int(cks[1, w]), int(cks[2, w])
                c_w = c1 + c2 + c3
                msg = mpool.tile([128, cmax, OUT_CH], fp16, tag="msg")
                parts = [(0, 0, c1), (G2BASE, c1, c2), (G3BASE, c1 + c2, c3)]
                for base, coff, ncols in parts:
                    for cc0 in range(0, ncols, GCHUNK_COLS):
                        cc = min(GCHUNK_COLS, ncols - cc0)
                        o16 = (o + coff + cc0) * 8
                        nc.gpsimd.dma_gather(
                            msg[:, coff + cc0:coff + cc0 + cc, :],
                            gtab[base:, :],
                            ix[:, o16:o16 + cc * 8],
                            128 * cc, 128 * cc, OUT_CH,
                            queue_num=gq % 4,
                            single_packet=False,
                        )
                        gq += 1
                rt = rpool.tile([128, OUT_CH], fp32, tag="rt")
                if c_w > 8:
                    # stage 1: contiguous fp16 pairwise add into f32
                    ch = c_w // 2
                    acc = apool.tile([128, (cmax + 1) // 2, OUT_CH], fp32,
                                     tag="acc")
                    nc.vector.tensor_tensor(
                        acc[:, 0:ch, :], msg[:, 0:ch, :],
                        msg[:, ch:2 * ch, :], op=mybir.AluOpType.add,
                    )
                    if c_w % 2:
                        nc.vector.tensor_copy(
                            acc[:, ch:ch + 1, :], msg[:, 2 * ch:c_w, :]
                        )
                        ch += 1
                    nc.vector.tensor_reduce(
                        rt[:],
                        acc[:, 0:ch, :].transpose([0, 2, 1]),
                        axis=mybir.AxisListType.X,
                        op=mybir.AluOpType.add,
                    )
                else:
                    nc.vector.tensor_reduce(
                        rt[:],
                        msg[:, 0:c_w, :].transpose([0, 2, 1]),
                        axis=mybir.AxisListType.X,
                        op=mybir.AluOpType.add,
                    )
                nc.vector.tensor_scalar_mul(rt[:], rt[:], dw[:, w:w + 1])
                nc.sync.dma_start(outd[w], rt[:])

    nc.compile()
    return nc


def _install_ntff_shim():
    """The agent image's antenv lacks axon_hooks; register a shim wired to
    the libaxon NTFF profiler so trace=True works. No-op when already
    importable or when profiling isn't possible."""
    import sys
    import types
    try:
        import antenv.axon_hooks  # noqa: F401
        return
    except ImportError:
        pass
    hook = None
    try:
        from trn_agent_boot.trn_boot import _ntff_profile_via_ctypes
        hook = _ntff_profile_via_ctypes("/opt/axon/libaxon_pjrt.so")
    except Exception:
        hook = None
    mod = types.ModuleType("antenv.axon_hooks")
    mod._hook = hook
    mod.get_axon_ntff_profile_hook = lambda: mod._hook
    def _set(h):
        mod._hook = h
    mod.set_axon_ntff_profile_hook = _set
    sys.modules["antenv.axon_hooks"] = mod
    try:
        import antenv
        antenv.axon_hooks = mod
    except Exception:
        pass


def kernel(x, edge_index, W, b):
    import os
    os.environ.setdefault("NEURON_RT_RESET_CORES", "1")
    x = np.asarray(x, dtype=np.float32)
    W = np.asarray(W, dtype=np.float32)
    b = np.asarray(b, dtype=np.float32)

    plan = _plan(edge_index)
    nc = _build_bass(plan["cks"], plan["totc"], plan["tot16"])

    xT_pad = np.zeros((IN_CH, NPAD), dtype=np.float16)
    xT_pad[:, :N_NODES] = x.T.astype(np.float16)
    wT = np.ascontiguousarray(W.T.astype(np.float16))
    bvec4 = np.ascontiguousarray(np.broadcast_to(
        np.tile(b.reshape(1, OUT_CH), (1, PGROUP)), (128, PGROUP * OUT_CH)
    ).astype(np.float32))
    dis_pad = np.zeros(NPAD, dtype=np.float32)
    dis_pad[:N_NODES] = plan["dis"]
    disg = np.ascontiguousarray(dis_pad.reshape(NT, 128).T)

    in_maps = []
    for c in range(N_CORES):
        in_maps.append({
            "xT": xT_pad,
            "wT": wT,
            "bvec4": bvec4,
            "disg": disg,
            "disw": np.ascontiguousarray(plan["disw"][c]),
            "idx": plan["idx16"][c],
        })

    _install_ntff_shim()
    from concourse.bass_utils import run_bass_kernel_spmd
    res = run_bass_kernel_spmd(nc, in_maps, core_ids=list(range(N_CORES)))
    globals()["_last_results"] = res

    out = np.empty((N_NODES, OUT_CH), dtype=np.float32)
    for c in range(N_CORES):
        outd = res.results[c]["outd"]
        gdest = plan["gdest"][c]
        mask = gdest >= 0
        out[gdest[mask]] = outd[mask]
    return out


# revision 30
# speedup vs baseline: 1.1101x; 1.1101x over previous
"""GCNConv on 8 Trainium2 NeuronCores.

out = D^-1/2 (A + I) D^-1/2 (x @ W.T + b)

Strategy (dest-bucket sharding):
  - algebra: norm[e] = dis[row]*dis[col] folds into node scalings:
        g = dis * (x @ W.T + b)          (per-node, on device)
        out[d] = dis[d] * sum_{e: row=d} g[col_e]
  - host: append self loops, bucket edges by dest core (6250 dests/core),
    sort dests by degree into windows of 128 (one dest per SBUF
    partition), build int16 gather grids for SWDGE dma_gather. int16
    indices only span 32768 table rows, so three overlapping base regions
    are used; each dest's edges are split ~evenly across the three grids
    (cols sorted ascending; contiguous ranges stay region-valid).
  - device (SPMD x8): phase A = g table via PE matmul (bias preloaded
    into PSUM by the scalar engine, 4 node tiles per PSUM bank, one DVE
    scale+cast per tile); phase B = dma_gather of g rows into
    [128, C_w, 128] tiles (one dest per partition), DVE reduce over the
    slot axis, scale by dis[dest], store.
  - host: inverse-permute the [win, part] layout back to node order.

g table layout (GT_ROWS = NPAD + 384):
  row 0..127            zeros
  row n+128             g[node n]          for n < 25024
  row 25152..25279      zeros
  row n+256             g[node n]          for n >= 25024
  row NPAD+256..+383    zeros
Gather base regions (int16 idx = row - base):
  G1 base 0     -> nodes <= 32511   (pad idx 0)
  G2 base 8832  -> nodes in [8704, 41343]   (pad idx 16320)
  G3 base 17664 -> nodes >= 17536   (pad idx 32640)
"""

import numpy as np

N_NODES = 50000
N_EDGES = 1600000
IN_CH = 256
OUT_CH = 128
N_CORES = 8

DPC = N_NODES // N_CORES          # dests per core
WPC = (DPC + 127) // 128          # windows per core
NPAD = ((N_NODES + 127) // 128) * 128   # padded node count
NT = NPAD // 128                  # node tiles for the matmul
GT_ROWS = NPAD + 384
MIDSPLIT = 25024                  # node where the middle zero band sits
G2BASE = 8832
G3BASE = 17664
G1MAX = 32511                     # max node reachable from base 0
G2MIN, G2MAX = 8704, 41343
G3MIN = 17536
G2PAD = 16320
G3PAD = 32640
GCHUNK_COLS = 16                  # dma_gather chunk: 16 cols * 128 = 2048 idxs
XSLAB_T = 23                      # node tiles per x slab (23*17 = 391 = NT)
PGROUP = 4                        # node tiles per PSUM bank


def _row_of(n):
    n = np.asarray(n, dtype=np.int64)
    return n + 128 + 128 * (n >= MIDSPLIT)


def _plan(edge_index):
    """Host-side index preprocessing. Returns per-core gather grids and the
    permutation needed to unshard."""
    ei0 = np.asarray(edge_index[0], dtype=np.int64)
    ei1 = np.asarray(edge_index[1], dtype=np.int64)
    self_idx = np.arange(N_NODES, dtype=np.int64)
    row = np.concatenate([ei0, self_idx])
    col = np.concatenate([ei1, self_idx])

    deg = np.bincount(row, minlength=N_NODES)
    n1o = np.bincount(row[col < G2MIN], minlength=N_NODES)
    n3o = np.bincount(row[col > G2MAX], minlength=N_NODES)
    m1 = np.bincount(row[col <= G1MAX], minlength=N_NODES)
    m3 = np.bincount(row[col >= G3MIN], minlength=N_NODES)
    dis = deg.astype(np.float32) ** -0.5

    # 3-way split of each dest's (ascending) cols: first k1 -> G1,
    # middle k2 -> G2, last k3 -> G3; clip keeps every range region-valid.
    k1 = np.clip((deg + 2) // 3, n1o, m1)
    k3 = np.clip((deg - k1 + 1) // 2, n3o, np.minimum(m3, deg - k1))
    k2 = deg - k1 - k3

    order = np.lexsort((col, row))
    col_sorted = col[order].astype(np.int64)
    row_sorted = _row_of(col_sorted)  # table rows, in per-dest ascending order
    starts = np.zeros(N_NODES + 1, dtype=np.int64)
    np.cumsum(deg, out=starts[1:])

    # per-core window assignment: lex sort by (max k, total degree) desc —
    # keeps all three per-window column maxima tight simultaneously
    maxk = np.maximum(np.maximum(k1, k2), k3)
    perms = []
    kmax_pc = np.zeros((3, N_CORES, WPC), dtype=np.int64)
    for c in range(N_CORES):
        sl = slice(c * DPC, (c + 1) * DPC)
        perm = np.lexsort((-deg[sl], -maxk[sl]))
        perms.append(perm)
        for j, kk in enumerate((k1, k2, k3)):
            ks = kk[sl][perm]
            for w in range(WPC):
                s = w * 128
                e = min(s + 128, DPC)
                kmax_pc[j, c, w] = ks[s:e].max() if s < DPC else 0
    cks = kmax_pc.max(axis=1)          # [3, WPC]
    cw = cks.sum(axis=0)
    cw = np.maximum(cw, 1)
    offs = np.zeros(WPC + 1, dtype=np.int64)
    np.cumsum(cw, out=offs[1:])
    totc = int(offs[-1])

    idx_arrs, disw_arrs, gdests = [], [], []
    for c in range(N_CORES):
        perm = perms[c]
        grid = np.zeros((128, totc), dtype=np.int16)
        disw = np.zeros((128, WPC), dtype=np.float32)
        gdest = np.full((WPC, 128), -1, dtype=np.int64)
        for w in range(WPC):
            o = int(offs[w])
            c1, c2, c3 = int(cks[0, w]), int(cks[1, w]), int(cks[2, w])
            grid[:, o + c1:o + c1 + c2] = G2PAD
            grid[:, o + c1 + c2:o + c1 + c2 + c3] = G3PAD
            for p in range(128):
                s = w * 128 + p
                if s >= DPC:
                    break
                d = c * DPC + int(perm[s])
                a1, a2, a3 = int(k1[d]), int(k2[d]), int(k3[d])
                st = int(starts[d])
                rows_d = row_sorted[st:st + a1 + a2 + a3]
                grid[p, o:o + a1] = rows_d[:a1].astype(np.int16)
                grid[p, o + c1:o + c1 + a2] = \
                    (rows_d[a1:a1 + a2] - G2BASE).astype(np.int16)
                grid[p, o + c1 + c2:o + c1 + c2 + a3] = \
                    (rows_d[a1 + a2:] - G3BASE).astype(np.int16)
                disw[p, w] = dis[d]
                gdest[w, p] = d
        idx_arrs.append(grid)
        disw_arrs.append(disw)
        gdests.append(gdest)

    # wrap grids into the dma_gather idx layout: logical slot j
    # (= col*128 + part) lives at [16*g + j%16, j//16] for each of the 8
    # replicated 16-partition groups.
    idx16 = []
    tot16 = totc * 8
    for c in range(N_CORES):
        L = idx_arrs[c].T.ravel()
        base = L.reshape(tot16, 16).T
        idx16.append(np.ascontiguousarray(np.tile(base, (8, 1))))

    return {
        "dis": dis,
        "cks": cks,
        "cw": cw.astype(np.int64),
        "totc": totc,
        "tot16": tot16,
        "idx16": idx16,
        "disw": disw_arrs,
        "gdest": gdests,
    }


def _build_bass(cks, totc, tot16):
    """Build the single SPMD Bass program (same NEFF on all 8 cores)."""
    import concourse.bacc as bacc
    import concourse.mybir as mybir
    import concourse.tile as tile
    from concourse.library_config import mlp

    fp32 = mybir.dt.float32
    fp16 = mybir.dt.float16
    i16 = mybir.dt.int16

    nc = bacc.Bacc(
        "TRN2",
        target_bir_lowering=False,
        dynamic_dma_scratch_size=65536,
        num_swdge_queues=4,
    )

    xT = nc.dram_tensor("xT", [IN_CH, NPAD], fp16, kind="ExternalInput")
    wT = nc.dram_tensor("wT", [IN_CH, OUT_CH], fp16, kind="ExternalInput")
    bvec4 = nc.dram_tensor("bvec4", [128, PGROUP * OUT_CH], fp32,
                           kind="ExternalInput")
    disg = nc.dram_tensor("disg", [128, NT], fp32, kind="ExternalInput")
    disw = nc.dram_tensor("disw", [128, WPC], fp32, kind="ExternalInput")
    idx = nc.dram_tensor("idx", [128, tot16], i16, kind="ExternalInput")

    gtab = nc.dram_tensor("gtab", [GT_ROWS, OUT_CH], fp16, kind="Internal")
    outd = nc.dram_tensor("outd", [WPC, 128, OUT_CH], fp32, kind="ExternalOutput")

    cw = cks.sum(axis=0)
    offs = np.zeros(len(cw) + 1, dtype=np.int64)
    np.cumsum(cw, out=offs[1:])
    cmax = int(max(cw))

    with tile.TileContext(nc) as tc:
        # ---------------- phase A: g = dis * (x @ W.T + b) ----------------
        with (
            tc.tile_pool(name="constA", bufs=1) as cpool,
            tc.tile_pool(name="xslab", bufs=2) as xpool,
            tc.tile_pool(name="gout", bufs=4) as gpool,
            tc.tile_pool(name="psum", bufs=4, space="PSUM") as ppool,
        ):
            nc.gpsimd.load_library(mlp)
            wt0 = cpool.tile([128, OUT_CH], fp16, tag="wt0")
            wt1 = cpool.tile([128, OUT_CH], fp16, tag="wt1")
            bt4 = cpool.tile([128, PGROUP * OUT_CH], fp32, tag="bt4")
            dg = cpool.tile([128, NT], fp32, tag="dg")
            nc.sync.dma_start(wt0[:], wT[0:128, :])
            nc.sync.dma_start(wt1[:], wT[128:256, :])
            nc.sync.dma_start(bt4[:], bvec4[:])
            nc.sync.dma_start(dg[:], disg[:])

            # zero rows absorb padding gathers
            zt = cpool.tile([128, 3 * OUT_CH], fp16, tag="zt")
            nc.vector.memset(zt[:], 0.0)
            nc.sync.dma_start(gtab[0:128, :], zt[:, 0:OUT_CH])
            nc.sync.dma_start(gtab[MIDSPLIT + 128:MIDSPLIT + 256, :],
                              zt[:, OUT_CH:2 * OUT_CH])
            nc.sync.dma_start(gtab[NPAD + 256:GT_ROWS, :],
                              zt[:, 2 * OUT_CH:])

            nslab = NT // XSLAB_T
            for s in range(nslab):
                c0 = s * XSLAB_T * 128
                cols = XSLAB_T * 128
                xa = xpool.tile([128, cols], fp16, tag="xa")
                xb = xpool.tile([128, cols], fp16, tag="xb")
                nc.sync.dma_start(xa[:], xT[0:128, c0:c0 + cols])
                nc.sync.dma_start(xb[:], xT[128:256, c0:c0 + cols])
                t = 0
                while t < XSLAB_T:
                    gn = min(PGROUP, XSLAB_T - t)
                    ps = ppool.tile([128, PGROUP, OUT_CH], fp32, tag="ps")
                    gt = gpool.tile([128, PGROUP, OUT_CH], fp16, tag="gt")
                    gs = gpool.tile([128, PGROUP, OUT_CH], fp32, tag="gs")
                    for j in range(gn):
                        gt_i = s * XSLAB_T + t + j
                        sl = slice((t + j) * 128, (t + j + 1) * 128)
                        nc.tensor.matmul(
                            ps[:, j, :], xa[:, sl], wt0[:],
                            start=True, stop=False,
                        )
                        nc.tensor.matmul(
                            ps[:, j, :], xb[:, sl], wt1[:],
                            start=False, stop=True,
                        )
                        nc.vector.tensor_tensor(
                            gs[:, j, :], ps[:, j, :], bt4[:, 0:OUT_CH],
                            op=mybir.AluOpType.add,
                        )
                        nc.vector.tensor_scalar_mul(
                            gt[:, j, :], gs[:, j, :], dg[:, gt_i:gt_i + 1]
                        )
                        r0 = int(_row_of(gt_i * 128))
                        r1 = int(_row_of(gt_i * 128 + 64))
                        if r1 == r0 + 64:
                            nc.sync.dma_start(
                                gtab[r0:r0 + 128, :], gt[:, j, :]
                            )
                        else:  # tile straddling the middle zero band
                            nc.sync.dma_start(
                                gtab[r0:r0 + 64, :], gt[0:64, j, :]
                            )
                            nc.sync.dma_start(
                                gtab[r1:r1 + 64, :], gt[64:128, j, :]
                            )
                    t += gn

        # ---------------- phase B: gather + segment reduce ----------------
        with (
            tc.tile_pool(name="constB", bufs=1) as bpool,
            tc.tile_pool(name="msg", bufs=4) as mpool,
            tc.tile_pool(name="accp", bufs=2) as apool,
            tc.tile_pool(name="red", bufs=4) as rpool,
        ):
            dw = bpool.tile([128, WPC], fp32, tag="dw")
            nc.sync.dma_start(dw[:], disw[:])
            ix = bpool.tile([128, tot16], i16, tag="ix")
            nc.sync.dma_start(ix[:], idx[:])
            gq = 0

            for w in range(WPC):
                o = int(offs[w])
                c1, c2, c3 = int(cks[0, w]), int(cks[1, w]), int(cks[2, w])
                c_w = c1 + c2 + c3
                msg = mpool.tile([128, cmax, OUT_CH], fp16, tag="msg")
                parts = [(0, 0, c1), (G2BASE, c1, c2), (G3BASE, c1 + c2, c3)]
                for base, coff, ncols in parts:
                    for cc0 in range(0, ncols, GCHUNK_COLS):
                        cc = min(GCHUNK_COLS, ncols - cc0)
                        o16 = (o + coff + cc0) * 8
                        nc.gpsimd.dma_gather(
                            msg[:, coff + cc0:coff + cc0 + cc, :],
                            gtab[base:, :],
                            ix[:, o16:o16 + cc * 8],
                            128 * cc, 128 * cc, OUT_CH,
                            queue_num=gq % 4,
                            single_packet=False,
                        )
                        gq += 1
                rt = rpool.tile([128, OUT_CH], fp32, tag="rt")
                if c_w > 8:
                    # stage 1: contiguous fp16 pairwise add into f32
                    ch = c_w // 2
                    acc = apool.tile([128, (cmax + 1) // 2, OUT_CH], fp32,
                                     tag="acc")
                    nc.vector.tensor_tensor(
                        acc[:, 0:ch, :], msg[:, 0:ch, :],
                        msg[:, ch:2 * ch, :], op=mybir.AluOpType.add,
                    )
                    if c_w % 2:
                        nc.vector.tensor_copy(
                            acc[:, ch:ch + 1, :], msg[:, 2 * ch:c_w, :]
                        )
                        ch += 1
                    nc.vector.tensor_reduce(
                        rt[:],
                        acc[:, 0:ch, :].transpose([0, 2, 1]),
                        axis=mybir.AxisListType.X,
                        op=mybir.AluOpType.add,
                    )
                else:
                    nc.vector.tensor_reduce(
                        rt[:],
                        msg[:, 0:c_w, :].transpose([0, 2, 1]),
                        axis=mybir.AxisListType.X,
                        op=mybir.AluOpType.add,
                    )
                rt2 = rpool.tile([128, OUT_CH], fp32, tag="rt2")
                nc.scalar.activation(
                    rt2[:], rt[:], mybir.ActivationFunctionType.Copy,
                    scale=dw[:, w:w + 1],
                )
                nc.sync.dma_start(outd[w], rt2[:])

    nc.compile()
    return nc


def _install_ntff_shim():
    """The agent image's antenv lacks axon_hooks; register a shim wired to
    the libaxon NTFF profiler so trace=True works. No-op when already
    importable or when profiling isn't possible."""
    import sys
    import types
    try:
        import antenv.axon_hooks  # noqa: F401
        return
    except ImportError:
        pass
    hook = None
    try:
        from trn_agent_boot.trn_boot import _ntff_profile_via_ctypes
        hook = _ntff_profile_via_ctypes("/opt/axon/libaxon_pjrt.so")
    except Exception:
        hook = None
    mod = types.ModuleType("antenv.axon_hooks")
    mod._hook = hook
    mod.get_axon_ntff_profile_hook = lambda: mod._hook
    def _set(h):
        mod._hook = h
    mod.set_axon_ntff_profile_hook = _set
    sys.modules["antenv.axon_hooks"] = mod
    try:
        import antenv
        antenv.axon_hooks = mod
    except Exception:
        pass


def kernel(x, edge_index, W, b):
    import os
    os.environ.setdefault("NEURON_RT_RESET_CORES", "1")
    x = np.asarray(x, dtype=np.float32)
    W = np.asarray(W, dtype=np.float32)
    b = np.asarray(b, dtype=np.float32)

    plan = _plan(edge_index)
    nc = _build_bass(plan["cks"], plan["totc"], plan["tot16"])

    xT_pad = np.zeros((IN_CH, NPAD), dtype=np.float16)
    xT_pad[:, :N_NODES] = x.T.astype(np.float16)
    wT = np.ascontiguousarray(W.T.astype(np.float16))
    bvec4 = np.ascontiguousarray(np.broadcast_to(
        np.tile(b.reshape(1, OUT_CH), (1, PGROUP)), (128, PGROUP * OUT_CH)
    ).astype(np.float32))
    dis_pad = np.zeros(NPAD, dtype=np.float32)
    dis_pad[:N_NODES] = plan["dis"]
    disg = np.ascontiguousarray(dis_pad.reshape(NT, 128).T)

    in_maps = []
    for c in range(N_CORES):
        in_maps.append({
            "xT": xT_pad,
            "wT": wT,
            "bvec4": bvec4,
            "disg": disg,
            "disw": np.ascontiguousarray(plan["disw"][c]),
            "idx": plan["idx16"][c],
        })

    _install_ntff_shim()
    from concourse.bass_utils import run_bass_kernel_spmd
    res = run_bass_kernel_spmd(nc, in_maps, core_ids=list(range(N_CORES)))
    globals()["_last_results"] = res

    out = np.empty((N_NODES, OUT_CH), dtype=np.float32)
    for c in range(N_CORES):
        outd = res.results[c]["outd"]
        gdest = plan["gdest"][c]
        mask = gdest >= 0
        out[gdest[mask]] = outd[mask]
    return out
